# revision 12
# baseline (speedup 1.0000x reference)
"""Trainium2 Bass kernel for per-pixel cosine-distance block.

x1: [B, C, h, w]  f32
x2: [B, S, C, h, w] f32
out: [B, S*h*w] f32  where out[b, s*h*w + p] = 1 - cos(x1[b,:,p], x2[b,s,:,p])
(cosine over the channel dim C, per pixel)

Sharding: data-parallel over B across 8 NeuronCores (4 batches per core).

Per-core algorithm (C=512 on partitions as 4 chunks of 128, hw=1024 on free):
  dot[s,hw] = sum_c x1[c,hw] * x2[s,c,hw]   (DVE mult, TensorE
                                             one-hot-matmul partition-reduce)
  ss2[s,hw] = sum_c x2[s,c,hw]^2            (ScalarE square, matmul)
  ss1[hw]   = sum_c x1[c,hw]^2              (from bf16 x1, DVE mult)
  out = 1 - dot * rsqrt(ss1) * rsqrt(ss2)

The kernel is HBM-bound (72 MiB of input per core, ~360-420 GB/s effective
with the 2-NC-per-stack QoS arbitration), so the structure keeps both DMA
paths saturated end to end while dodging their pathologies:
  - x2 tiles are split between the Sync HWDGE ring (f32) and the SWDGE ring
    (inline f32->bf16 cast). SDMA engine 15 runs ~10% slower on SWDGE
    traffic (descriptor-ring port contention), so SWDGE tiles are biased
    early in the run; its straggler finishes under the HWDGE tail.
  - x1 is cast-loaded bf16 via SWDGE once per batch, interleaved between x2
    loads; the bf16 copy pairs with bf16 x2 tiles (2x DVE) and with f32
    tiles via mixed-dtype multiply.
  - ss1 / rsqrt(ss1) / its broadcast across the S rows are computed in
    per-batch prologue chunks emitted inside the previous batch's s-loop
    (negated via a minus-ones lhsT), leaving a short per-batch epilogue:
    rsqrt(ss2) -> two DVE mults -> ACT Copy(+1.0) -> store. The epilogue is
    split per PSUM bank so it overlaps the tail matmuls, and the epilogue of
    batch b is emitted inside batch b+1's s-loop.
  - output stores ride the Scalar HWDGE ring, issued right after the ACT
    that produces them, so no DMA issue ever head-of-line-blocks a queue.
"""

from contextlib import ExitStack

import numpy as np

import concourse.bass as bass
import concourse.tile as tile
from concourse import bacc, mybir
from concourse.bass_utils import run_bass_kernel_spmd

B, S, C, H, W = 32, 8, 512, 32, 32
HW = H * W  # 1024
N_CORES = 8
BL = B // N_CORES  # 4 batches per core
P = 128
NCH = C // P  # 4 chunks of the channel dim
HWH = HW // 2  # 512 (one PSUM bank of f32)

FP32 = mybir.dt.float32
BF16 = mybir.dt.bfloat16

# PSUM accumulator row layout (quadrant-based so engines can read each group)
D_ROW = 0  # rows 0..S-1: dot[s]
S2_ROW = 32  # rows 32..32+S-1: ss2[s]
NR = S2_ROW + S

# x2 tile routing by flat index t = b*S + s: SWDGE (bf16 cast) for an early-
# biased third, Sync HWDGE (f32) for the rest. Engine 15's SWDGE straggle
# then lands inside the HWDGE tail instead of extending the kernel.
SWDGE_TILE = [(t % 3 == 1 and t < 30) for t in range(BL * S)]


def _emit(ctx: ExitStack, tc: tile.TileContext, x1, x2, out):
    nc = tc.nc

    # c = k*128 + p  ->  partition p, chunk k
    x1r = x1.rearrange("b (k p) f -> b p k f", p=P)  # [BL, 128, NCH, HW]
    x2r = x2.rearrange("b s (k p) f -> b s p k f", p=P)  # [BL, S, 128, NCH, HW]

    singles = ctx.enter_context(tc.tile_pool(name="singles", bufs=1))
    x2b_pool = ctx.enter_context(tc.tile_pool(name="x2b", bufs=3))
    x2f_pool = ctx.enter_context(tc.tile_pool(name="x2f", bufs=3))
    prod_pool = ctx.enter_context(tc.tile_pool(name="prod", bufs=3))
    sq1_pool = ctx.enter_context(tc.tile_pool(name="sq1", bufs=1))
    rr1_pool = ctx.enter_context(tc.tile_pool(name="rr1p", bufs=1))
    sq2_pool = ctx.enter_context(tc.tile_pool(name="sq2", bufs=2))
    ep_pool = ctx.enter_context(tc.tile_pool(name="ep", bufs=2))
    out_pool = ctx.enter_context(tc.tile_pool(name="outp", bufs=2))
    psum_pool = ctx.enter_context(tc.tile_pool(name="pacc", bufs=2, space="PSUM"))
    ps1_pool = ctx.enter_context(tc.tile_pool(name="ps1", bufs=1, space="PSUM"))
    prep_pool = ctx.enter_context(tc.tile_pool(name="prep", bufs=1, space="PSUM"))

    # oh8[:, s, :] is a [P, S] matrix that is all-ones in column s, zero
    # elsewhere: matmul with it as lhsT deposits the partition-reduction of
    # rhs into PSUM row s of the output slice and adds zero to the others.
    oh8 = singles.tile([P, S, S], BF16)
    nc.vector.memset(oh8, 0.0)
    for s in range(S):
        nc.vector.memset(oh8[:, s, s : s + 1], 1.0)
    ones1 = singles.tile([P, 1], BF16)
    nc.vector.memset(ones1, 1.0)

    # [1, S] minus-ones: K=1 matmul replicates an SBUF row across S PSUM
    # partitions, negated — bakes the final `1 - x` sign into rsqrt(ss1).
    mones_row = singles.tile([1, S], FP32)
    nc.vector.memset(mones_row, -1.0)

    # bf16 copy of all four x1 batches (cast during the SWDGE load)
    x1b = singles.tile([P, BL, NCH, HW], BF16)
    # -rsqrt(ss1) replicated across the S rows, one tile per batch (engine
    # APs must start at a partition quadrant, so no packing across batches)
    r1n = [singles.tile([S, 2, HWH], FP32, name=f"r1n{b}") for b in range(BL)]

    def emit_prologue_chunk(b):
        # ss1[b] -> -rsqrt(ss1[b]) broadcast to the S rows of r1n[b]
        sq1 = sq1_pool.tile([P, NCH, HW], BF16)
        nc.vector.tensor_mul(sq1[:], x1b[:, b], x1b[:, b])
        ps1 = ps1_pool.tile([1, 2, HWH], FP32)
        for hh in range(2):
            for ic in range(NCH):
                nc.tensor.matmul(
                    ps1[:, hh, :],
                    ones1,
                    sq1[:, ic, hh * HWH : (hh + 1) * HWH],
                    start=(ic == 0),
                    stop=(ic == NCH - 1),
                )
        rr1 = rr1_pool.tile([1, 2, HWH], FP32)
        nc.scalar.activation(
            rr1[:], ps1[:], func=mybir.ActivationFunctionType.Abs_reciprocal_sqrt
        )
        prep = prep_pool.tile([S, 2, HWH], FP32)
        for hh in range(2):
            nc.tensor.matmul(
                prep[:, hh, :], mones_row, rr1[:, hh, :], start=True, stop=True
            )
        nc.vector.tensor_copy(r1n[b][:], prep[:])

    def emit_epilogue(pall, b):
        # out[b] = 1 + dot * rsqrt(ss2) * (-rsqrt(ss1)), one PSUM bank at
        # a time so bank hh=0's chain overlaps bank hh=1's matmuls
        for hh in range(2):
            rr2 = ep_pool.tile([S, HWH], FP32)
            nc.scalar.activation(
                rr2[:],
                pall[S2_ROW : S2_ROW + S, hh, :],
                func=mybir.ActivationFunctionType.Abs_reciprocal_sqrt,
            )
            rr12n = ep_pool.tile([S, HWH], FP32)
            nc.vector.tensor_mul(rr12n[:], rr2[:], r1n[b][:, hh, :])
            t2 = ep_pool.tile([S, HWH], FP32)
            nc.vector.tensor_mul(t2[:], pall[D_ROW : D_ROW + S, hh, :], rr12n[:])
            dist = out_pool.tile([S, HWH], FP32)
            nc.scalar.activation(
                dist[:], t2[:], func=mybir.ActivationFunctionType.Copy, bias=1.0
            )
            # Scalar (ACT) HWDGE issue directly after the ACT that wrote
            # `dist`: the data dependency is already satisfied in queue
            # order, so this never stalls the Scalar queue.
            nc.scalar.dma_start(out[b, :, hh * HWH : (hh + 1) * HWH], dist[:])

    # first batch's x1, cast-loaded ahead of the x2 stream
    nc.gpsimd.dma_start(x1b[:, 0], x1r[0])

    pending = None
    for b in range(BL):
        pall = psum_pool.tile([NR, 2, HWH], FP32)  # 2 banks

        for s in range(S):
            t = b * S + s
            if SWDGE_TILE[t]:
                x2_t = x2b_pool.tile([P, NCH, HW], BF16)
                nc.gpsimd.dma_start(x2_t[:], x2r[b, s])  # f32 -> bf16 cast
            else:
                x2_t = x2f_pool.tile([P, NCH, HW], FP32)
                nc.sync.dma_start(x2_t[:], x2r[b, s])

            # next batch's x1 cast rides the SWDGE ring mid-batch
            if s == 3 and b + 1 < BL:
                nc.gpsimd.dma_start(x1b[:, b + 1], x1r[b + 1])

            prod = prod_pool.tile([P, NCH, HW], BF16)
            nc.vector.tensor_mul(prod[:], x1b[:, b], x2_t[:])
            sq2 = sq2_pool.tile([P, NCH, HW], BF16)
            nc.scalar.activation(
                sq2[:], x2_t[:], func=mybir.ActivationFunctionType.Square
            )

            for hh in range(2):
                # ss2 before dot: rsqrt(ss2) is the longer epilogue chain
                for ic in range(NCH):
                    nc.tensor.matmul(
                        pall[S2_ROW : S2_ROW + S, hh, :],
                        oh8[:, s, :],
                        sq2[:, ic, hh * HWH : (hh + 1) * HWH],
                        start=(s == 0 and ic == 0),
                        stop=(s == S - 1 and ic == NCH - 1),
                    )
                for ic in range(NCH):
                    nc.tensor.matmul(
                        pall[D_ROW : D_ROW + S, hh, :],
                        oh8[:, s, :],
                        prod[:, ic, hh * HWH : (hh + 1) * HWH],
                        start=(s == 0 and ic == 0),
                        stop=(s == S - 1 and ic == NCH - 1),
                    )

            if b == 0 and s == 0:
                emit_prologue_chunk(0)
            # previous batch's epilogue, pipelined into this batch's s-loop
            if s == 1 and pending is not None:
                emit_epilogue(*pending)
                pending = None
            # next batch's ss1 pipeline, once its x1 cast has landed
            if s == 6 and b + 1 < BL:
                emit_prologue_chunk(b + 1)

        pending = (pall, b)

    emit_epilogue(*pending)


def _build():
    # Bacc (not plain Bass): its compile pipeline legalizes TRN2's
    # one-sync-wait-per-instruction limit (generate_event_semaphores).
    nc = bacc.Bacc("TRN2")
    x1 = nc.dram_tensor("x1", [BL, C, HW], FP32, kind="ExternalInput")
    x2 = nc.dram_tensor("x2", [BL, S, C, HW], FP32, kind="ExternalInput")
    out = nc.dram_tensor("out", [BL, S, HW], FP32, kind="ExternalOutput")
    with tile.TileContext(nc) as tc:
        with ExitStack() as ctx:
            _emit(ctx, tc, x1[:], x2[:], out[:])
    nc.finalize()
    return nc


_NC = None

# test-harness knobs (the grading harness never touches these)
TRACE = False
TRACE_DIR = None
LAST_RESULTS = None


def _get_nc():
    global _NC
    if _NC is None:
        _NC = _build()
    return _NC


def kernel(x1: np.ndarray, x2: np.ndarray) -> np.ndarray:
    global LAST_RESULTS
    x1 = np.ascontiguousarray(x1, dtype=np.float32).reshape(B, C, HW)
    x2 = np.ascontiguousarray(x2, dtype=np.float32).reshape(B, S, C, HW)
    nc = _get_nc()
    in_maps = [
        {"x1": x1[c * BL : (c + 1) * BL], "x2": x2[c * BL : (c + 1) * BL]}
        for c in range(N_CORES)
    ]
    res = run_bass_kernel_spmd(
        nc, in_maps, list(range(N_CORES)), trace=TRACE, tmpdir=TRACE_DIR
    )
    LAST_RESULTS = res
    outs = [res.results[c]["out"].reshape(BL, S * HW) for c in range(N_CORES)]
    return np.concatenate(outs, axis=0)


# revision 13
# speedup vs baseline: 1.0336x; 1.0336x over previous
"""Trainium2 Bass kernel for per-pixel cosine-distance block.

x1: [B, C, h, w]  f32
x2: [B, S, C, h, w] f32
out: [B, S*h*w] f32  where out[b, s*h*w + p] = 1 - cos(x1[b,:,p], x2[b,s,:,p])
(cosine over the channel dim C, per pixel)

Sharding: data-parallel over B across 8 NeuronCores (4 batches per core).

Per-core algorithm (C=512 on partitions as 4 chunks of 128, hw=1024 on free):
  dot[s,hw] = sum_c x1[c,hw] * x2[s,c,hw]   (DVE mult, TensorE
                                             one-hot-matmul partition-reduce)
  ss2[s,hw] = sum_c x2[s,c,hw]^2            (ScalarE square, matmul)
  ss1[hw]   = sum_c x1[c,hw]^2              (from bf16 x1, DVE mult)
  out = 1 - dot * rsqrt(ss1) * rsqrt(ss2)

The kernel is HBM-bound (72 MiB of input per core, ~360-420 GB/s effective
with the 2-NC-per-stack QoS arbitration), so the structure keeps both DMA
paths saturated end to end while dodging their pathologies:
  - x2 tiles are split between the Sync HWDGE ring (f32) and the SWDGE ring
    (inline f32->bf16 cast). SDMA engine 15 runs ~10% slower on SWDGE
    traffic (descriptor-ring port contention), so SWDGE tiles are biased
    early in the run; its straggler finishes under the HWDGE tail.
  - x1 is cast-loaded bf16 via SWDGE once per batch, interleaved between x2
    loads; the bf16 copy pairs with bf16 x2 tiles (2x DVE) and with f32
    tiles via mixed-dtype multiply.
  - ss1 / rsqrt(ss1) / its broadcast across the S rows are computed in
    per-batch prologue chunks emitted inside the previous batch's s-loop
    (negated via a minus-ones lhsT), leaving a short per-batch epilogue:
    rsqrt(ss2) -> two DVE mults -> ACT Copy(+1.0) -> store. The epilogue is
    split per PSUM bank so it overlaps the tail matmuls, and the epilogue of
    batch b is emitted inside batch b+1's s-loop.
  - output stores ride the Scalar HWDGE ring, issued right after the ACT
    that produces them, so no DMA issue ever head-of-line-blocks a queue.
"""

from contextlib import ExitStack

import numpy as np

import concourse.bass as bass
import concourse.tile as tile
from concourse import bacc, mybir
from concourse.bass_utils import run_bass_kernel_spmd

B, S, C, H, W = 32, 8, 512, 32, 32
HW = H * W  # 1024
N_CORES = 8
BL = B // N_CORES  # 4 batches per core
P = 128
NCH = C // P  # 4 chunks of the channel dim
HWH = HW // 2  # 512 (one PSUM bank of f32)

FP32 = mybir.dt.float32
BF16 = mybir.dt.bfloat16

# PSUM accumulator row layout (quadrant-based so engines can read each group)
D_ROW = 0  # rows 0..S-1: dot[s]
S2_ROW = 32  # rows 32..32+S-1: ss2[s]
NR = S2_ROW + S

# x2 tile routing by flat index t = b*S + s: SWDGE (bf16 cast) for a third,
# Sync HWDGE (f32) for the rest. The spread keeps both DMA paths pulling the
# whole run; the last two tiles stay on HWDGE so SDMA engine 15's ~10% SWDGE
# straggle (descriptor-ring port contention) lands inside the HWDGE tail.
SWDGE_TILE = [(t % 3 == 2) for t in range(BL * S)]


def _emit(ctx: ExitStack, tc: tile.TileContext, x1, x2, out):
    nc = tc.nc

    # c = k*128 + p  ->  partition p, chunk k
    x1r = x1.rearrange("b (k p) f -> b p k f", p=P)  # [BL, 128, NCH, HW]
    x2r = x2.rearrange("b s (k p) f -> b s p k f", p=P)  # [BL, S, 128, NCH, HW]

    singles = ctx.enter_context(tc.tile_pool(name="singles", bufs=1))
    x2b_pool = ctx.enter_context(tc.tile_pool(name="x2b", bufs=3))
    x2f_pool = ctx.enter_context(tc.tile_pool(name="x2f", bufs=4))
    prod_pool = ctx.enter_context(tc.tile_pool(name="prod", bufs=3))
    sq1_pool = ctx.enter_context(tc.tile_pool(name="sq1", bufs=1))
    rr1_pool = ctx.enter_context(tc.tile_pool(name="rr1p", bufs=1))
    sq2_pool = ctx.enter_context(tc.tile_pool(name="sq2", bufs=2))
    ep_pool = ctx.enter_context(tc.tile_pool(name="ep", bufs=2))
    out_pool = ctx.enter_context(tc.tile_pool(name="outp", bufs=2))
    psum_pool = ctx.enter_context(tc.tile_pool(name="pacc", bufs=2, space="PSUM"))
    ps1_pool = ctx.enter_context(tc.tile_pool(name="ps1", bufs=1, space="PSUM"))
    prep_pool = ctx.enter_context(tc.tile_pool(name="prep", bufs=1, space="PSUM"))

    # oh8[:, s, :] is a [P, S] matrix that is all-ones in column s, zero
    # elsewhere: matmul with it as lhsT deposits the partition-reduction of
    # rhs into PSUM row s of the output slice and adds zero to the others.
    oh8 = singles.tile([P, S, S], BF16)
    nc.vector.memset(oh8, 0.0)
    for s in range(S):
        nc.vector.memset(oh8[:, s, s : s + 1], 1.0)
    ones1 = singles.tile([P, 1], BF16)
    nc.vector.memset(ones1, 1.0)

    # [1, S] minus-ones: K=1 matmul replicates an SBUF row across S PSUM
    # partitions, negated — bakes the final `1 - x` sign into rsqrt(ss1).
    mones_row = singles.tile([1, S], FP32)
    nc.vector.memset(mones_row, -1.0)

    # bf16 copy of all four x1 batches (cast during the SWDGE load)
    x1b = singles.tile([P, BL, NCH, HW], BF16)
    # -rsqrt(ss1) replicated across the S rows, one tile per batch (engine
    # APs must start at a partition quadrant, so no packing across batches)
    r1n = [singles.tile([S, 2, HWH], BF16, name=f"r1n{b}") for b in range(BL)]

    def emit_prologue_chunk(b):
        # ss1[b] -> -rsqrt(ss1[b]) broadcast to the S rows of r1n[b]
        sq1 = sq1_pool.tile([P, NCH, HW], BF16)
        nc.vector.tensor_mul(sq1[:], x1b[:, b], x1b[:, b])
        ps1 = ps1_pool.tile([1, 2, HWH], FP32)
        for hh in range(2):
            for ic in range(NCH):
                nc.tensor.matmul(
                    ps1[:, hh, :],
                    ones1,
                    sq1[:, ic, hh * HWH : (hh + 1) * HWH],
                    start=(ic == 0),
                    stop=(ic == NCH - 1),
                )
        rr1 = rr1_pool.tile([1, 2, HWH], FP32)
        nc.scalar.activation(
            rr1[:], ps1[:], func=mybir.ActivationFunctionType.Abs_reciprocal_sqrt
        )
        prep = prep_pool.tile([S, 2, HWH], FP32)
        for hh in range(2):
            nc.tensor.matmul(
                prep[:, hh, :], mones_row, rr1[:, hh, :], start=True, stop=True
            )
        nc.vector.tensor_copy(r1n[b][:], prep[:])

    def emit_epilogue(pall, b):
        # out[b] = 1 + dot * rsqrt(ss2) * (-rsqrt(ss1)), one PSUM bank at
        # a time so bank hh=0's chain overlaps bank hh=1's matmuls
        for hh in range(2):
            rr2 = ep_pool.tile([S, HWH], FP32)
            nc.scalar.activation(
                rr2[:],
                pall[S2_ROW : S2_ROW + S, hh, :],
                func=mybir.ActivationFunctionType.Abs_reciprocal_sqrt,
            )
            rr12n = ep_pool.tile([S, HWH], FP32)
            nc.vector.tensor_mul(rr12n[:], rr2[:], r1n[b][:, hh, :])
            t2 = ep_pool.tile([S, HWH], FP32)
            nc.vector.tensor_mul(t2[:], pall[D_ROW : D_ROW + S, hh, :], rr12n[:])
            dist = out_pool.tile([S, HWH], FP32)
            nc.scalar.activation(
                dist[:], t2[:], func=mybir.ActivationFunctionType.Copy, bias=1.0
            )
            # Scalar (ACT) HWDGE issue directly after the ACT that wrote
            # `dist`: the data dependency is already satisfied in queue
            # order, so this never stalls the Scalar queue.
            nc.scalar.dma_start(out[b, :, hh * HWH : (hh + 1) * HWH], dist[:])

    # first batch's x1, cast-loaded ahead of the x2 stream
    nc.gpsimd.dma_start(x1b[:, 0], x1r[0])

    pending = None
    for b in range(BL):
        pall = psum_pool.tile([NR, 2, HWH], FP32)  # 2 banks

        for s in range(S):
            t = b * S + s
            if SWDGE_TILE[t]:
                x2_t = x2b_pool.tile([P, NCH, HW], BF16)
                nc.gpsimd.dma_start(x2_t[:], x2r[b, s])  # f32 -> bf16 cast
            else:
                x2_t = x2f_pool.tile([P, NCH, HW], FP32)
                nc.sync.dma_start(x2_t[:], x2r[b, s])

            # next batch's x1 cast rides the SWDGE ring mid-batch
            if s == 3 and b + 1 < BL:
                nc.gpsimd.dma_start(x1b[:, b + 1], x1r[b + 1])

            prod = prod_pool.tile([P, NCH, HW], BF16)
            nc.vector.tensor_mul(prod[:], x1b[:, b], x2_t[:])
            sq2 = sq2_pool.tile([P, NCH, HW], BF16)
            nc.scalar.activation(
                sq2[:], x2_t[:], func=mybir.ActivationFunctionType.Square
            )

            for hh in range(2):
                # ss2 before dot: rsqrt(ss2) is the longer epilogue chain
                for ic in range(NCH):
                    nc.tensor.matmul(
                        pall[S2_ROW : S2_ROW + S, hh, :],
                        oh8[:, s, :],
                        sq2[:, ic, hh * HWH : (hh + 1) * HWH],
                        start=(s == 0 and ic == 0),
                        stop=(s == S - 1 and ic == NCH - 1),
                    )
                for ic in range(NCH):
                    nc.tensor.matmul(
                        pall[D_ROW : D_ROW + S, hh, :],
                        oh8[:, s, :],
                        prod[:, ic, hh * HWH : (hh + 1) * HWH],
                        start=(s == 0 and ic == 0),
                        stop=(s == S - 1 and ic == NCH - 1),
                    )

            if b == 0 and s == 0:
                emit_prologue_chunk(0)
            # previous batch's epilogue, pipelined into this batch's s-loop
            if s == 1 and pending is not None:
                emit_epilogue(*pending)
                pending = None
            # next batch's ss1 pipeline, once its x1 cast has landed
            if s == 6 and b + 1 < BL:
                emit_prologue_chunk(b + 1)

        pending = (pall, b)

    emit_epilogue(*pending)


def _build():
    # Bacc (not plain Bass): its compile pipeline legalizes TRN2's
    # one-sync-wait-per-instruction limit (generate_event_semaphores).
    nc = bacc.Bacc("TRN2")
    x1 = nc.dram_tensor("x1", [BL, C, HW], FP32, kind="ExternalInput")
    x2 = nc.dram_tensor("x2", [BL, S, C, HW], FP32, kind="ExternalInput")
    out = nc.dram_tensor("out", [BL, S, HW], FP32, kind="ExternalOutput")
    with tile.TileContext(nc) as tc:
        with ExitStack() as ctx:
            _emit(ctx, tc, x1[:], x2[:], out[:])
    nc.finalize()
    return nc


_NC = None

# test-harness knobs (the grading harness never touches these)
TRACE = False
TRACE_DIR = None
LAST_RESULTS = None


def _get_nc():
    global _NC
    if _NC is None:
        _NC = _build()
    return _NC


def kernel(x1: np.ndarray, x2: np.ndarray) -> np.ndarray:
    global LAST_RESULTS
    x1 = np.ascontiguousarray(x1, dtype=np.float32).reshape(B, C, HW)
    x2 = np.ascontiguousarray(x2, dtype=np.float32).reshape(B, S, C, HW)
    nc = _get_nc()
    in_maps = [
        {"x1": x1[c * BL : (c + 1) * BL], "x2": x2[c * BL : (c + 1) * BL]}
        for c in range(N_CORES)
    ]
    res = run_bass_kernel_spmd(
        nc, in_maps, list(range(N_CORES)), trace=TRACE, tmpdir=TRACE_DIR
    )
    LAST_RESULTS = res
    outs = [res.results[c]["out"].reshape(BL, S * HW) for c in range(N_CORES)]
    return np.concatenate(outs, axis=0)
